# revision 11
# baseline (speedup 1.0000x reference)
"""LogicGatedSNN Trainium2 kernel.

Full (unsharded) inputs in, full outputs out. Internally shards out_features
(rows) across 8 NeuronCores; spike_input is broadcast. No collectives.

Per-core program (O_SH = 1024 rows):
  phase 0: load x, broadcast to 128 partitions via ones-matmul on PE.
  phase 1: stream synapse tiles [128, CH]; one fused DVE tensor_scalar
           (is_gt 50, accum) -> conn partials; one fused DVE
           scalar_tensor_tensor ((s > 50) * X, accum) -> current partials.
  epilogue (per group of row-blocks, [128, G] vectors): conn/current
           reduction, f = 15*rsqrt(max(conn,5)) via ACT sqrt + DVE recip +
           2x Newton, v = 0.85*m + current, spikes, new_thr, new_v.
  phase 2: trace rows = X_bcast * spikes[p] on ACT (Copy with per-partition
           scale); general path additionally streams the old trace and does
           (T * 0.9) + SX then clip via fused DVE ops.
  vectors are PE-transposed to [24, 128] so the [1024] outputs are written
  with contiguous descriptors.
"""

import numpy as np

import concourse.bacc as bacc
import concourse.bass as bass
import concourse.mybir as mybir
from concourse.bass_utils import run_bass_kernel_spmd
from concourse.tile import TileContext

F32 = mybir.dt.float32
OP = mybir.AluOpType
AF = mybir.ActivationFunctionType

N_CORES = 8
O_FULL = 8192
I_FULL = 8192
O_SH = O_FULL // N_CORES  # 1024 rows per core
P = 128
NJ = O_SH // P  # 8 row blocks per core
THRESH = 50.0

_CACHE = {}
LAST_EXEC_NS = None
LAST_PROFILE = None


def _install_ntff_hook():
    """Provide antenv.axon_hooks (missing in this image) so trace=True works."""
    import sys
    import types

    if "antenv.axon_hooks" in sys.modules:
        return True
    try:
        from trn_agent_boot.trn_boot import _ntff_profile_via_ctypes

        hook = _ntff_profile_via_ctypes("/opt/axon/libaxon_pjrt.so")
        if hook is None:
            return False
    except Exception:
        return False
    mod = types.ModuleType("antenv.axon_hooks")
    state = {"hook": hook}
    mod.get_axon_ntff_profile_hook = lambda: state["hook"]
    mod.set_axon_ntff_profile_hook = lambda h: state.update(hook=h)
    sys.modules["antenv.axon_hooks"] = mod
    # keep artifacts local; no fish/S3 share in this container
    import concourse.bass_utils as bu

    bu.upload_artifacts = lambda tmpdir: tmpdir
    return True


def _build(general: bool, ch: int, groups: int):
    """Build the per-core Bass program. general=True streams the old trace."""
    nc = bacc.Bacc("TRN2", target_bir_lowering=False)

    x_ext = nc.declare_dram_parameter("x", [I_FULL], F32, isOutput=False)
    syn_ext = nc.declare_dram_parameter("syn", [O_SH, I_FULL], F32, isOutput=False)
    mem_ext = nc.declare_dram_parameter("mem", [O_SH], F32, isOutput=False)
    thr_ext = nc.declare_dram_parameter("thr", [O_SH], F32, isOutput=False)
    eye_ext = nc.declare_dram_parameter("eye", [P, P], F32, isOutput=False)
    if general:
        tr_ext = nc.declare_dram_parameter("tr", [O_SH, I_FULL], F32, isOutput=False)
    spk_ext = nc.declare_dram_parameter("spk_out", [O_SH], F32, isOutput=True)
    vm_ext = nc.declare_dram_parameter("vmem_out", [O_SH], F32, isOutput=True)
    nthr_ext = nc.declare_dram_parameter("thr_out", [O_SH], F32, isOutput=True)
    trace_ext = nc.declare_dram_parameter(
        "trace_out", [O_SH, I_FULL], F32, isOutput=True
    )

    nch = I_FULL // ch  # chunks per row block
    jg = NJ // groups  # row blocks per epilogue group

    with TileContext(nc) as tc:
        with (
            tc.tile_pool(name="const", bufs=1) as const_pool,
            tc.tile_pool(name="vec", bufs=1) as vec_pool,
            tc.tile_pool(name="syn", bufs=3) as syn_pool,
            tc.tile_pool(name="tr", bufs=3 if general else 1) as tr_pool,
            tc.tile_pool(name="out", bufs=3) as out_pool,
            tc.tile_pool(name="scratch", bufs=1) as scratch_pool,
            tc.tile_pool(name="psum", bufs=2, space="PSUM") as psum_pool,
        ):
            # ---- phase 0: constants ----
            x_row = const_pool.tile([1, I_FULL], F32, tag="xrow")
            nc.sync.dma_start(out=x_row[:], in_=x_ext[:].rearrange("(a i) -> a i", a=1))
            eye_t = const_pool.tile([P, P], F32, tag="eye")
            nc.sync.dma_start(out=eye_t[:], in_=eye_ext[:])
            ones_t = const_pool.tile([1, P], F32, tag="ones")
            nc.vector.memset(ones_t[:], 1.0)

            xb = const_pool.tile([P, I_FULL], F32, tag="xb")
            for n0 in range(0, I_FULL, 512):
                ps = psum_pool.tile([P, 512], F32, tag="ps")
                nc.tensor.matmul(
                    ps[:], ones_t[:], x_row[:, n0 : n0 + 512], start=True, stop=True
                )
                nc.scalar.copy(xb[:, n0 : n0 + 512], ps[:])

            # per-core vectors, [128, NJ] layout: row 128*j + p -> [p, j]
            mem_t = vec_pool.tile([P, NJ], F32, tag="mem")
            nc.sync.dma_start(
                out=mem_t[:], in_=mem_ext[:].rearrange("(j p) -> p j", p=P)
            )
            thr_t = vec_pool.tile([P, NJ], F32, tag="thr")
            nc.sync.dma_start(
                out=thr_t[:], in_=thr_ext[:].rearrange("(j p) -> p j", p=P)
            )

            conn_parts = vec_pool.tile([P, NJ * nch], F32, tag="connp")
            cur_parts = vec_pool.tile([P, NJ * nch], F32, tag="curp")
            conn4 = vec_pool.tile([P, NJ], F32, tag="conn4")
            cur4 = vec_pool.tile([P, NJ], F32, tag="cur4")
            sq8 = vec_pool.tile([P, NJ], F32, tag="sq8")
            y8 = vec_pool.tile([P, NJ], F32, tag="y8")
            t8 = vec_pool.tile([P, NJ], F32, tag="t8")
            v8 = vec_pool.tile([P, NJ], F32, tag="v8")
            # packed outputs: cols 0:8 spikes, 8:16 new_v, 16:24 new_thr
            vecs = vec_pool.tile([P, 3 * NJ], F32, tag="vecs")
            w_scr = scratch_pool.tile([P, ch], F32, tag="wscr")

            for g in range(groups):
                # ---- phase 1: masked matvec + row sums ----
                for j in range(g * jg, (g + 1) * jg):
                    r0 = j * P
                    for c in range(nch):
                        c0 = c * ch
                        idx = j * nch + c
                        s_tile = syn_pool.tile([P, ch], F32, tag="stile")
                        nc.sync.dma_start(
                            out=s_tile[:], in_=syn_ext[r0 : r0 + P, c0 : c0 + ch]
                        )
                        # w = (s > 50); accum -> conn partial
                        nc.vector.tensor_scalar(
                            w_scr[:],
                            s_tile[:],
                            THRESH,
                            None,
                            OP.is_gt,
                            OP.add,
                            accum_out=conn_parts[:, idx : idx + 1],
                        )
                        # wx = (s > 50) * x; accum -> current partial
                        nc.vector.scalar_tensor_tensor(
                            w_scr[:],
                            s_tile[:],
                            THRESH,
                            xb[:, c0 : c0 + ch],
                            OP.is_gt,
                            OP.mult,
                            accum_out=cur_parts[:, idx : idx + 1],
                        )

                # ---- epilogue for this group ----
                S = slice(g * jg, (g + 1) * jg)
                nc.vector.tensor_reduce(
                    conn4[:, S],
                    conn_parts[:, g * jg * nch : (g + 1) * jg * nch].rearrange(
                        "p (j c) -> p j c", c=nch
                    ),
                    axis=mybir.AxisListType.X,
                    op=OP.add,
                )
                nc.vector.tensor_reduce(
                    cur4[:, S],
                    cur_parts[:, g * jg * nch : (g + 1) * jg * nch].rearrange(
                        "p (j c) -> p j c", c=nch
                    ),
                    axis=mybir.AxisListType.X,
                    op=OP.add,
                )
                # conn = max(conn, 5)
                nc.vector.tensor_scalar(conn4[:, S], conn4[:, S], 5.0, None, OP.max)
                # y ~= rsqrt(conn), 2x Newton refinement
                nc.scalar.activation(sq8[:, S], conn4[:, S], AF.Sqrt)
                nc.vector.reciprocal(y8[:, S], sq8[:, S])
                for _ in range(2):
                    nc.vector.tensor_tensor(t8[:, S], y8[:, S], y8[:, S], OP.mult)
                    nc.vector.tensor_tensor(t8[:, S], t8[:, S], conn4[:, S], OP.mult)
                    nc.vector.tensor_scalar(
                        t8[:, S], t8[:, S], -0.5, 1.5, OP.mult, OP.add
                    )
                    nc.vector.tensor_tensor(y8[:, S], y8[:, S], t8[:, S], OP.mult)
                # current = cur * y * 15
                nc.vector.tensor_tensor(cur4[:, S], cur4[:, S], y8[:, S], OP.mult)
                nc.vector.tensor_scalar(cur4[:, S], cur4[:, S], 15.0, None, OP.mult)
                # v = 0.85*m + current
                nc.vector.scalar_tensor_tensor(
                    v8[:, S], mem_t[:, S], 0.85, cur4[:, S], OP.mult, OP.add
                )
                # spikes = (v >= thr)
                spkS = vecs[:, g * jg : (g + 1) * jg]
                nc.vector.tensor_tensor(spkS, v8[:, S], thr_t[:, S], OP.is_ge)
                # new_thr = clip(thr + 0.1*spk - 0.01, 2, 15)
                nc.vector.scalar_tensor_tensor(
                    t8[:, S], spkS, 0.1, thr_t[:, S], OP.mult, OP.add
                )
                nc.vector.tensor_scalar(t8[:, S], t8[:, S], -0.01, 15.0, OP.add, OP.min)
                nc.vector.tensor_scalar(
                    vecs[:, 2 * NJ + g * jg : 2 * NJ + (g + 1) * jg],
                    t8[:, S],
                    2.0,
                    None,
                    OP.max,
                )
                # new_v = v * (0.1 - 0.1*spk)
                nc.vector.tensor_scalar(t8[:, S], spkS, -0.1, 0.1, OP.mult, OP.add)
                nc.vector.tensor_tensor(
                    vecs[:, NJ + g * jg : NJ + (g + 1) * jg], v8[:, S], t8[:, S], OP.mult
                )

                # ---- phase 2: trace rows for this group ----
                for j in range(g * jg, (g + 1) * jg):
                    r0 = j * P
                    for c in range(nch):
                        c0 = c * ch
                        t_tile = out_pool.tile([P, ch], F32, tag="ttile")
                        if general:
                            sx_tile = out_pool.tile([P, ch], F32, tag="sxtile")
                            nc.scalar.activation(
                                sx_tile[:],
                                xb[:, c0 : c0 + ch],
                                AF.Copy,
                                bias=0.0,
                                scale=vecs[:, j : j + 1],
                            )
                            tr_tile = tr_pool.tile([P, ch], F32, tag="trtile")
                            nc.sync.dma_start(
                                out=tr_tile[:], in_=tr_ext[r0 : r0 + P, c0 : c0 + ch]
                            )
                            nc.vector.scalar_tensor_tensor(
                                t_tile[:], tr_tile[:], 0.9, sx_tile[:], OP.mult, OP.add
                            )
                            nc.vector.tensor_scalar(
                                t_tile[:], t_tile[:], 0.0, 5.0, OP.max, OP.min
                            )
                        else:
                            nc.scalar.activation(
                                t_tile[:],
                                xb[:, c0 : c0 + ch],
                                AF.Copy,
                                bias=0.0,
                                scale=vecs[:, j : j + 1],
                            )
                        nc.sync.dma_start(
                            out=trace_ext[r0 : r0 + P, c0 : c0 + ch], in_=t_tile[:]
                        )

            # ---- vector outputs: transpose to [24, 128], contiguous stores ----
            psT = psum_pool.tile([3 * NJ, P], F32, tag="psT")
            nc.tensor.transpose(psT[:], vecs[:], eye_t[:])
            vecsT = vec_pool.tile([3 * NJ, P], F32, tag="vecsT")
            nc.scalar.copy(vecsT[:], psT[:])
            nc.sync.dma_start(
                out=spk_ext[:].rearrange("(j p) -> j p", p=P), in_=vecsT[0:NJ, :]
            )
            nc.sync.dma_start(
                out=vm_ext[:].rearrange("(j p) -> j p", p=P),
                in_=vecsT[NJ : 2 * NJ, :],
            )
            nc.sync.dma_start(
                out=nthr_ext[:].rearrange("(j p) -> j p", p=P),
                in_=vecsT[2 * NJ : 3 * NJ, :],
            )

    nc.compile()
    return nc


def kernel(
    spike_input,
    synapse_states,
    membrane_potential,
    adaptive_threshold,
    eligibility_trace,
    _trace=False,
):
    global LAST_EXEC_NS, LAST_PROFILE
    x = np.ascontiguousarray(np.asarray(spike_input, dtype=np.float32))
    syn = np.ascontiguousarray(np.asarray(synapse_states, dtype=np.float32))
    mem = np.ascontiguousarray(np.asarray(membrane_potential, dtype=np.float32))
    thr = np.ascontiguousarray(np.asarray(adaptive_threshold, dtype=np.float32))
    tr = np.ascontiguousarray(np.asarray(eligibility_trace, dtype=np.float32))

    general = bool(tr.any())
    key = (general,)
    if key not in _CACHE:
        if general:
            _CACHE[key] = _build(general=True, ch=2048, groups=2)
        else:
            _CACHE[key] = _build(general=False, ch=4096, groups=2)
    nc = _CACHE[key]

    eye = np.eye(P, dtype=np.float32)
    in_maps = []
    for k in range(N_CORES):
        r0 = k * O_SH
        m = {
            "x": x,
            "syn": syn[r0 : r0 + O_SH],
            "mem": mem[r0 : r0 + O_SH],
            "thr": thr[r0 : r0 + O_SH],
            "eye": eye,
        }
        if general:
            m["tr"] = tr[r0 : r0 + O_SH]
        in_maps.append(m)

    kwargs = {}
    if _trace and _install_ntff_hook():
        import tempfile

        kwargs = {"trace": True, "tmpdir": tempfile.mkdtemp(prefix="bass_prof_")}
    res = run_bass_kernel_spmd(nc, in_maps, list(range(N_CORES)), **kwargs)
    LAST_EXEC_NS = res.exec_time_ns
    LAST_PROFILE = res.profile_json

    spikes = np.concatenate([np.asarray(res.results[k]["spk_out"]) for k in range(N_CORES)])
    new_v = np.concatenate([np.asarray(res.results[k]["vmem_out"]) for k in range(N_CORES)])
    new_thr = np.concatenate([np.asarray(res.results[k]["thr_out"]) for k in range(N_CORES)])
    new_trace = np.concatenate(
        [np.asarray(res.results[k]["trace_out"]) for k in range(N_CORES)], axis=0
    )
    return spikes, new_v, new_thr, new_trace
